# revision 1
# baseline (speedup 1.0000x reference)
"""DifColorQuantization Trainium2 kernel, v2.

Math (per pixel p, codebook color k):
    ref:  argmin_k sqrt(sum_c (x_c - cb_kc + eps)^2 + eps) ; out = cb[argmin]
    sqrt/+eps are monotone, so rank by the k-dependent part of the expanded
    square:  s_k = sum_c w_kc * x_c + b_k,  w_kc = 2*(eps-cb_kc),
    b_k = sum_c (eps-cb_kc)^2  (the sum_c x_c^2 term is k-independent).

v2 changes vs v1:
  - scores: single bf16 matmul per block with a 49-row split
    lhsT=[x_hi;x_lo;x_hi;x_lo;1], rhs=[w_hi;w_hi;w_lo;w_lo;b] (fp32 PSUM
    accumulation) instead of fp32 matmuls: 4x fewer PE cycles, ~1e-7 score
    error (flips ~100 near-tie pixels of 1M; host-measured rel-l2 2.2e-3).
  - gather: 1-term bf16 codebook (output = bf16-rounded colors, rel 1.6e-3)
    instead of exact 3-term split: 3x fewer PE cycles; y output in bf16.
  - score bias split b_hi+b_lo over two ones-rows so the bf16 weight
    storage stays fp32-accurate (single bf16 bias flips ~5% of pixels).
  - reduce + is_equal on DVE; PSUM evictions on ACT (GPSIMD/Pool cannot
    access PSUM).  Measured rel-l2 error 3.1e-3 vs the fp32 reference.
"""

import numpy as np

H = 1024
W = 1024
K = 32
EPS = 1e-6
NCORES = 8
ROWS = H // NCORES            # 128 rows per core
NPX = ROWS * W                # 131072 pixels per core
TILE_PX = 2048                # pixels per tile (4 slots x 512)
NSLOT = 4
SLOT_N = 512                  # columns per slot
NT = NPX // TILE_PX           # 64 tiles



def _build_program(n_tiles, reps=1):
    import concourse.bass as bass
    import concourse.bacc as bacc
    import concourse.tile as tile
    from concourse import mybir

    f32 = mybir.dt.float32
    bf16 = mybir.dt.bfloat16

    nc = bacc.Bacc(None, target_bir_lowering=False)
    # x rows: [x_hi(12); x_lo(12); x_hi(12); x_lo(12); ones(2)] bf16,
    # rows 4c+q within each 12-group. col 512t+n <-> pixel 2048t+512q+n.
    # Two ones rows: the score bias is split b_hi+b_lo across them so it
    # stays fp32-accurate despite bf16 weight storage.
    L = SLOT_N * n_tiles
    x = nc.dram_tensor("x", [50, L], bf16, kind="ExternalInput")
    # bf16 consts: cols [0:128] iden, [128:256] wbd50 (rows 0-48),
    # [256:268] gather codebook gbd [128,12]
    cb16 = nc.dram_tensor("cb16", [128, 268], bf16, kind="ExternalInput")
    y = nc.dram_tensor("y", [12, L], bf16, kind="ExternalOutput")

    assert n_tiles % 2 == 0
    n_super = n_tiles // 2
    SUP = 2 * SLOT_N  # 1024 cols per supertile, 2 PSUM banks
    with tile.TileContext(nc) as tc:
        with (
            tc.tile_pool(name="const", bufs=1) as constp,
            tc.tile_pool(name="io", bufs=1) as iop,
            tc.tile_pool(name="work", bufs=4) as workp,
            tc.tile_pool(name="ps", bufs=2, space=bass.MemorySpace.PSUM) as psp,
            tc.tile_pool(name="pso", bufs=2, space=bass.MemorySpace.PSUM) as psop,
            tc.tile_pool(name="psq", bufs=2, space=bass.MemorySpace.PSUM) as psq,
        ):
            cons_t = constp.tile([128, 268], bf16)
            nc.sync.dma_start(cons_t[:], cb16[:])
            iden_t = cons_t[:, 0:128]
            wbd_t = cons_t[0:50, 128:256]
            gbd_t = cons_t[:, 256:268]

            img = iop.tile([50, L], bf16, tag="img")
            nc.sync.dma_start(img[:], x[:])

            def _body():
                for s in range(n_super):
                    _super(s)

            def _super(s):
                # transposed scores with bias: 8 blocks of [128 px, (q,k)]
                ps_T = psp.tile([128, SUP], f32, tag="ps_T")
                for b in range(8):
                    col = SUP * s + 128 * b
                    nc.tensor.matmul(
                        ps_T[:, 128 * b : 128 * (b + 1)],
                        img[:, col : col + 128],
                        wbd_t,
                    )

                # per-pixel min over the 32 scores (DVE)
                m = workp.tile([128, 32], f32, tag="m")
                nc.vector.tensor_reduce(
                    m[:],
                    ps_T[:].rearrange("p (s k) -> p s k", k=K),
                    axis=mybir.AxisListType.X,
                    op=mybir.AluOpType.min,
                )

                # one-hot in transposed layout; m broadcast along k via a
                # zero-stride AP (DVE)
                onehot = workp.tile([128, SUP], bf16, tag="onehot")
                nc.vector.tensor_tensor(
                    onehot[:].rearrange("p (s k) -> p s k", k=K),
                    ps_T[:].rearrange("p (s k) -> p s k", k=K),
                    m[:].to_broadcast((128, 32, K)),
                    op=mybir.AluOpType.is_equal,
                )

                # transpose back to [(q,k), px] per block
                ps_O = psop.tile([128, SUP], bf16, tag="ps_O")
                for b in range(8):
                    nc.tensor.transpose(
                        ps_O[:, 128 * b : 128 * (b + 1)],
                        onehot[:, 128 * b : 128 * (b + 1)],
                        iden_t,
                    )
                oh_sb = workp.tile([128, SUP], bf16, tag="oh_sb")
                nc.scalar.activation(
                    oh_sb[:], ps_O[:], mybir.ActivationFunctionType.Copy
                )

                # gather colors [12 (4c+q), 1024]: bf16 codebook, one matmul
                # per half; each half gets its own PSUM bank from a 2-ring,
                # evicted by ACT.
                o_sb = workp.tile([12, SUP], bf16, tag="o_sb")
                for h in range(2):
                    ps_o = psq.tile([12, SLOT_N], f32, tag="ps_o")
                    nc.tensor.matmul(
                        ps_o[:],
                        gbd_t,
                        oh_sb[:, SLOT_N * h : SLOT_N * (h + 1)],
                    )
                    nc.scalar.activation(
                        o_sb[:, SLOT_N * h : SLOT_N * (h + 1)],
                        ps_o[:],
                        mybir.ActivationFunctionType.Copy,
                    )

                nc.sync.dma_start(y[:, SUP * s : SUP * (s + 1)], o_sb[:])

            if reps == 1:
                _body()
            else:
                # hardware loop: used only for timing (program size stays
                # constant while the iteration count varies)
                with tc.For_i(0, reps, 1):
                    _body()
    nc.compile()
    return nc


def _host_consts(printability_array):
    """Pack kernel constants into one [128, 268] bf16 array.

    cols [0:128] identity, [128:256] score weights wbd50 (rows 0-48),
    [256:268] gather codebook.
    """
    import ml_dtypes

    cb = printability_array.reshape(K, 3).astype(np.float64)
    w = (2.0 * (EPS - cb)).astype(np.float32)            # [K, 3]
    b = np.sum((EPS - cb) ** 2, axis=1).astype(np.float32)  # [K]
    cbf = printability_array.reshape(K, 3).astype(np.float32)

    bf = ml_dtypes.bfloat16
    w_hi = w.astype(bf).astype(np.float32)
    w_lo = (w - w_hi).astype(bf).astype(np.float32)
    b_hi = b.astype(bf).astype(np.float32)
    b_lo = (b - b_hi).astype(bf).astype(np.float32)

    consts = np.zeros((128, 268), np.float32)
    consts[:, 0:128] = np.eye(128, dtype=np.float32)
    for q in range(NSLOT):
        for k in range(K):
            p = 32 * q + k
            consts[48, 128 + p] = b_hi[k]                # bias rows
            consts[49, 128 + p] = b_lo[k]
            for c in range(3):
                consts[12 * 0 + 4 * c + q, 128 + p] = w_hi[k, c]
                consts[12 * 1 + 4 * c + q, 128 + p] = w_hi[k, c]
                consts[12 * 2 + 4 * c + q, 128 + p] = w_lo[k, c]
                consts[12 * 3 + 4 * c + q, 128 + p] = w_lo[k, c]
            for c in range(3):
                consts[p, 256 + 4 * c + q] = cbf[k, c]   # gather codebook
    return consts.astype(bf)


_PROG_CACHE = {}


def _pack_x(flat3):
    """[3, npx] -> [50, npx/4] bf16: [x_hi;x_lo;x_hi;x_lo;1] blocks of
    rows 4c+q in (c, q, t, n) order."""
    import ml_dtypes

    bf = ml_dtypes.bfloat16
    npx = flat3.shape[1]
    nt = npx // TILE_PX
    v = flat3.reshape(3, nt, NSLOT, SLOT_N)          # (c, t, q, n)
    x12 = v.transpose(0, 2, 1, 3).reshape(12, nt * SLOT_N)
    x_hi = x12.astype(bf)
    x_lo = (x12 - x_hi.astype(np.float32)).astype(bf)
    out = np.empty((50, nt * SLOT_N), bf)
    out[0:12] = x_hi
    out[12:24] = x_lo
    out[24:36] = x_hi
    out[36:48] = x_lo
    out[48] = bf(1.0)
    out[49] = bf(1.0)
    return out


def _unpack_y(y12):
    """[12, npx/4] -> [3, npx] inverse of the image packing."""
    nt = y12.shape[1] // SLOT_N
    v = y12.astype(np.float32).reshape(3, NSLOT, nt, SLOT_N)  # (c, q, t, n)
    return v.transpose(0, 2, 1, 3).reshape(3, nt * TILE_PX)


def kernel(adv_patch, printability_array):
    from concourse.bass_utils import run_bass_kernel_spmd

    adv_patch = np.ascontiguousarray(adv_patch, dtype=np.float32)
    consts = _host_consts(np.asarray(printability_array, dtype=np.float32))

    if NT not in _PROG_CACHE:
        _PROG_CACHE[NT] = _build_program(NT)
    nc = _PROG_CACHE[NT]

    in_maps = []
    for i in range(NCORES):
        xs = adv_patch[:, i * ROWS : (i + 1) * ROWS, :].reshape(3, NPX)
        in_maps.append({"x": _pack_x(xs), "cb16": consts})

    res = run_bass_kernel_spmd(nc, in_maps, list(range(NCORES)))

    out = np.empty((1, 3, H, W), np.float32)
    for i in range(NCORES):
        out[0, :, i * ROWS : (i + 1) * ROWS, :] = _unpack_y(
            res.results[i]["y"]
        ).reshape(3, ROWS, W)
    return out

